# revision 2
# baseline (speedup 1.0000x reference)
"""Trainium2 Bass kernel for CrossAttention (silu-scored, masked) sharded over
8 NeuronCores.

Problem (full shapes):
    query/key/value: [2, 2048, 1024] f32, mask: [2, 1, 2048, 2048] int32
    Wq/Wk/Wv/Wo: [1024, 1024] f32, bq/bk/bv: [1024] f32
    out = silu(mask((q @ k.T) * scale)) @ v heads-merged @ Wo.T

Sharding: core c handles batch b = c // 4 and heads 4*(c%4) .. 4*(c%4)+3
(data parallel on B, tensor parallel on heads).  Each core computes a
row-parallel partial of the O-projection; the host sums the 4 partials per
batch.  No cross-device communication.

v2 design (vs the padded baseline):
  * score matmuls run the two heads of a pair CONCURRENTLY on the PE via
    row tiling (K=64 tiles at array rows 0-63 / 64-127, auto-derived from
    the operands' base partitions) -- no zero-padding of k, 2x PE win.
  * ctx matmuls pack the pair via col tiling (M=64 -> PSUM partitions
    0-63 / 64-127 of one shared accumulator) -- no M padding, and the
    accumulator IS the O-proj lhsT layout, so no scatter copies.
  * v is projected directly in [sk, dh] orientation (lhsT = x tile,
    rhs = Wv) -- kills the PE transposes + scatter of the baseline.
    v bias is folded in with a K=1 ones-matmul into the PSUM accumulator.
  * mask is resident in SBUF (loaded once, 8 MB) instead of streamed
    4x (32 MB); silu(masked) == mask-mul after silu since silu(0)=0 and
    the reference's -1e9 fill makes silu ~0 too.
  * mask applied AFTER silu, as bf16 x bf16 tensor_tensor (2x DVE mode).
  * SCALE folded into Wq/bq on the host.
  * attention walks sq in halves of 1024 so PSUM fits:
    sT pair 2x[128,1024] (4 banks) + acc [128,1024] (2) + pot 2x[128,512]
    (2) = 8 banks; O-projection of each half overlaps the next half's
    attention.
"""

import numpy as np
import ml_dtypes

B = 2
S = 2048
HID = 1024
HEADS = 16
DH = 64
N_CORES = 8
GROUPS = 4          # head-groups (cores per batch)
NH_LOC = HEADS // GROUPS  # 4 heads per core
DLOC = NH_LOC * DH        # 256 local features
SCALE = DH ** -0.5

F32 = np.float32
BF16 = ml_dtypes.bfloat16

_COMPILED = {}


def build_program():
    import concourse.bass as bass
    import concourse.tile as tile
    from concourse import bacc, mybir

    f32 = mybir.dt.float32
    bf16 = mybir.dt.bfloat16

    nc = bacc.Bacc("TRN2", target_bir_lowering=False, debug=False,
                   enable_asserts=False, num_devices=N_CORES)

    xq = nc.dram_tensor("xq", [HID, S], bf16, kind="ExternalInput").ap()
    xk = nc.dram_tensor("xk", [HID, S], bf16, kind="ExternalInput").ap()
    xv = nc.dram_tensor("xv", [HID, S], bf16, kind="ExternalInput").ap()
    mk = nc.dram_tensor("mk", [S, S], bf16, kind="ExternalInput").ap()
    wq = nc.dram_tensor("wq", [HID, DLOC], bf16, kind="ExternalInput").ap()
    wk = nc.dram_tensor("wk", [HID, DLOC], bf16, kind="ExternalInput").ap()
    wv = nc.dram_tensor("wv", [HID, DLOC], bf16, kind="ExternalInput").ap()
    wo = nc.dram_tensor("wo", [DLOC, HID], bf16, kind="ExternalInput").ap()
    bq = nc.dram_tensor("bq", [DLOC, 1], f32, kind="ExternalInput").ap()
    bk = nc.dram_tensor("bk", [DLOC, 1], f32, kind="ExternalInput").ap()
    bv = nc.dram_tensor("bv", [1, DLOC], bf16, kind="ExternalInput").ap()
    out1 = nc.dram_tensor("out1", [S, HID], bf16, kind="ExternalOutput").ap()

    SILU = mybir.ActivationFunctionType.Silu
    ADD = mybir.AluOpType.add

    with tile.TileContext(nc) as tc:
        with (
            tc.tile_pool(name="res", bufs=1) as res,
            tc.tile_pool(name="io", bufs=16) as io,
            tc.tile_pool(name="wp", bufs=3) as wp,
            # PSUM: sT 2x[128,1024] (4 banks) + acc 1x[128,1024] (2) +
            # pot 2x[128,512] (2) = 8 banks.  Phase A paccs ride the pot
            # slots; phase C pots do too (disjoint in time).
            tc.tile_pool(name="ps", bufs=2, space="PSUM") as ps,
            tc.tile_pool(name="attp", bufs=4) as attp,
            tc.tile_pool(name="oev", bufs=4) as oev,
        ):
            # ---- resident SBUF tensors ----
            qt = [res.tile([128, S], bf16, tag=f"qt{m}", name=f"qt{m}") for m in range(2)]
            kt2 = [res.tile([128, S], bf16, tag=f"kt{m}", name=f"kt{m}") for m in range(2)]
            ctxt = [res.tile([128, S], bf16, tag=f"ctxt{m}", name=f"ctxt{m}") for m in range(2)]
            v_sb = res.tile([128, 16 * DLOC], bf16, tag="vsb", name="v_sb")  # [sk%128, j*256+dh4]
            mk_sb = res.tile([128, 16 * S], bf16, tag="mk", name="mk_sb")    # [sk%128, j*2048+sq]
            wo_sb = [res.tile([128, HID], bf16, tag=f"wo{k}", name=f"wo_sb{k}") for k in range(2)]
            b_sb = {}
            for nm, srcb in (("bq", bq), ("bk", bk)):
                b_sb[nm] = [res.tile([128, 1], f32, tag=f"{nm}{m}", name=f"{nm}_sb{m}") for m in range(2)]
                for m in range(2):
                    nc.scalar.dma_start(out=b_sb[nm][m][:, :], in_=srcb[m * 128:(m + 1) * 128, :])
            bv_sb = res.tile([1, DLOC], bf16, tag="bv", name="bv_sb")
            nc.scalar.dma_start(out=bv_sb[:, :], in_=bv[:, :])
            ones = res.tile([1, 128], bf16, tag="ones", name="ones")
            nc.gpsimd.memset(ones[:, :], 1.0)
            scr = res.tile([1, 1], bf16, tag="scr", name="scr")
            # dummy silu: hoists the ACT table load out of the hot loop
            nc.scalar.activation(scr[:, :], ones[0:1, 0:1], SILU)
            for k in range(2):
                nc.scalar.dma_start(out=wo_sb[k][:, :], in_=wo[k * 128:(k + 1) * 128, :])

            # ---- input DMAs (two HWDGE queues: sync for k/q, scalar for rest) ----
            w_sb = {}
            for nm, w_ap in (("k", wk), ("q", wq), ("v", wv)):
                w_sb[nm] = wp.tile([128, 8 * DLOC], bf16, tag="w", name=f"w_{nm}")
                dmae = nc.sync if nm in ("k", "q") else nc.scalar
                dmae.dma_start(
                    out=w_sb[nm][:, :].rearrange("p (k m) -> p k m", k=8),
                    in_=w_ap.rearrange("(k p) m -> p k m", p=128),
                )
            xt = {}
            for nm, x_ap in (("k", xk), ("q", xq)):
                xt[nm] = []
                for k in range(8):
                    t = io.tile([128, S], bf16, tag="x", name=f"x_{nm}{k}")
                    nc.sync.dma_start(out=t[:, :], in_=x_ap[k * 128:(k + 1) * 128, :])
                    xt[nm].append(t)
            for j in range(4):
                nc.scalar.dma_start(out=mk_sb[:, j * S:(j + 1) * S], in_=mk[j * 128:(j + 1) * 128, :])
            xt["v"] = []
            for k in range(8):
                t = io.tile([128, S], bf16, tag="x", name=f"x_v{k}")
                nc.scalar.dma_start(out=t[:, :], in_=xv[k * 128:(k + 1) * 128, :])
                xt["v"].append(t)
            for j in range(4, 16):
                nc.scalar.dma_start(out=mk_sb[:, j * S:(j + 1) * S], in_=mk[j * 128:(j + 1) * 128, :])

            # ---- Phase A: projections ----
            def qk_proj(nm, dst, bias, nblocks):
                # both m per n-block so xk/xq tiles free up after one pass
                for n in nblocks:
                    pacc = {m: ps.tile([128, 512], f32, tag="pot", name=f"p{nm}{m}", bufs=2) for m in range(2)}
                    for k in range(8):
                        for m in range(2):
                            nc.tensor.matmul(
                                pacc[m][:, :],
                                lhsT=w_sb[nm][:, k * DLOC + m * 128: k * DLOC + (m + 1) * 128],
                                rhs=xt[nm][k][:, n * 512:(n + 1) * 512],
                                start=(k == 0), stop=(k == 7),
                            )
                    for m in range(2):
                        nc.vector.tensor_scalar(
                            out=dst[m][:, n * 512:(n + 1) * 512],
                            in0=pacc[m][:, :],
                            scalar1=bias[m][:, 0:1],
                            scalar2=None,
                            op0=ADD,
                        )

            def v_proj(jblocks):
                for j in jblocks:
                    pv = ps.tile([128, 512], f32, tag="pot", name=f"pv{j}", bufs=2)
                    nc.tensor.matmul(
                        pv[:, 0:DLOC], lhsT=ones[:, :], rhs=bv_sb[:, :],
                        start=True, stop=False,
                    )
                    for k in range(8):
                        nc.tensor.matmul(
                            pv[:, 0:DLOC],
                            lhsT=xt["v"][k][:, j * 128:(j + 1) * 128],
                            rhs=w_sb["v"][:, k * DLOC:(k + 1) * DLOC],
                            start=False, stop=(k == 7),
                        )
                    nc.vector.tensor_copy(
                        out=v_sb[:, j * DLOC:(j + 1) * DLOC], in_=pv[:, 0:DLOC])

            qk_proj("k", kt2, b_sb["bk"], range(4))
            qk_proj("q", qt, b_sb["bq"], range(2))
            v_proj(range(8))
            qk_proj("q", qt, b_sb["bq"], range(2, 4))
            v_proj(range(8, 16))

            # ---- Phase B+C: attention (sq halves), O-proj per half ----
            for half in range(2):
                sqo = half * 1024
                for t in range(2):
                    acc = ps.tile([128, 1024], f32, tag="acc", name=f"acc{half}{t}", bufs=1)
                    for j in range(16):
                        for h2 in range(2):
                            sT = ps.tile([128, 1024], f32, tag="sT", name="sT", bufs=2)
                            hsl = slice(h2 * 64, (h2 + 1) * 64)
                            for s2 in range(2):
                                nc.tensor.matmul(
                                    sT[:, s2 * 512:(s2 + 1) * 512],
                                    lhsT=kt2[t][hsl, j * 128:(j + 1) * 128],
                                    rhs=qt[t][hsl, sqo + s2 * 512: sqo + (s2 + 1) * 512],
                                    start=True, stop=True,
                                )
                            att = attp.tile([128, 1024], bf16, tag="att", name="att")
                            nc.scalar.activation(att[:, :], sT[:, :], SILU)
                            attn = attp.tile([128, 1024], bf16, tag="attn", name="attn")
                            nc.vector.tensor_mul(
                                out=attn[:, :], in0=att[:, :],
                                in1=mk_sb[:, j * S + sqo: j * S + sqo + 1024],
                            )
                            h = 2 * t + h2
                            for s2 in range(2):
                                nc.tensor.matmul(
                                    acc[hsl, s2 * 512:(s2 + 1) * 512],
                                    lhsT=v_sb[:, j * DLOC + h * 64: j * DLOC + (h + 1) * 64],
                                    rhs=attn[:, s2 * 512:(s2 + 1) * 512],
                                    start=(j == 0), stop=(j == 15),
                                )
                    nc.vector.tensor_copy(
                        out=ctxt[t][:, sqo:sqo + 1024], in_=acc[:, :])
                # O-projection for this half (overlaps next half's attention)
                for blk in range(half * 8, half * 8 + 8):
                    for n2 in range(2):
                        pot = ps.tile([128, 512], f32, tag="pot", name="pot", bufs=2)
                        for ki in range(2):
                            nc.tensor.matmul(
                                pot[:, :],
                                lhsT=ctxt[ki][:, blk * 128:(blk + 1) * 128],
                                rhs=wo_sb[ki][:, n2 * 512:(n2 + 1) * 512],
                                start=(ki == 0), stop=(ki == 1),
                            )
                        ev = oev.tile([128, 512], bf16, tag="oev", name="ev")
                        if half == 0 or n2 == 0:
                            nc.vector.tensor_copy(out=ev[:, :], in_=pot[:, :])
                        else:
                            nc.scalar.copy(out=ev[:, :], in_=pot[:, :])
                        nc.sync.dma_start(
                            out=out1[blk * 128:(blk + 1) * 128, n2 * 512:(n2 + 1) * 512],
                            in_=ev[:, :],
                        )

    nc.compile()
    return nc


def get_program():
    if "nc" not in _COMPILED:
        _COMPILED["nc"] = build_program()
    return _COMPILED["nc"]


def make_in_maps(query, key, value, mask, Wq, bq, Wk, bk, Wv, bv, Wo):
    """Host-side sharding/layout prep: one input map per core."""
    query = np.asarray(query, dtype=F32)
    key = np.asarray(key, dtype=F32)
    value = np.asarray(value, dtype=F32)
    mask = np.asarray(mask)
    in_maps = []
    maskT = [np.ascontiguousarray(mask[b, 0].T).astype(BF16) for b in range(B)]
    xqT = [np.ascontiguousarray(query[b].T).astype(BF16) for b in range(B)]
    xkT = [np.ascontiguousarray(key[b].T).astype(BF16) for b in range(B)]
    xvT = [np.ascontiguousarray(value[b].T).astype(BF16) for b in range(B)]
    for c in range(N_CORES):
        b = c // GROUPS
        g = c % GROUPS
        rs = slice(g * DLOC, (g + 1) * DLOC)
        in_maps.append({
            "xq": xqT[b],
            "xk": xkT[b],
            "xv": xvT[b],
            "mk": maskT[b],
            "wq": np.ascontiguousarray(SCALE * np.asarray(Wq, F32)[rs, :].T).astype(BF16),
            "wk": np.ascontiguousarray(np.asarray(Wk, F32)[rs, :].T).astype(BF16),
            "wv": np.ascontiguousarray(np.asarray(Wv, F32)[rs, :].T).astype(BF16),
            "wo": np.ascontiguousarray(np.asarray(Wo, F32)[:, rs].T).astype(BF16),
            "bq": (SCALE * np.asarray(bq, F32)[rs]).reshape(DLOC, 1),
            "bk": np.asarray(bk, F32)[rs].reshape(DLOC, 1),
            "bv": np.asarray(bv, F32)[rs].reshape(1, DLOC).astype(BF16),
        })
    return in_maps


def run_on_device(in_maps, trace=False, tmpdir=None):
    from concourse.bass_utils import run_bass_kernel_spmd
    nc = get_program()
    kwargs = {}
    if trace:
        kwargs.update(trace=True, tmpdir=tmpdir)
    return run_bass_kernel_spmd(nc, in_maps, list(range(N_CORES)), **kwargs)


def assemble_output(results):
    out = np.zeros((B, S, HID), dtype=F32)
    for c in range(N_CORES):
        out[c // GROUPS] += results[c]["out1"].astype(F32)
    return out


def kernel(query, key, value, mask, Wq, bq, Wk, bk, Wv, bv, Wo):
    in_maps = make_in_maps(query, key, value, mask, Wq, bq, Wk, bk, Wv, bv, Wo)
    res = run_on_device(in_maps)
    return assemble_output(res.results)


# revision 4
# speedup vs baseline: 1.2704x; 1.2704x over previous
"""Trainium2 Bass kernel for CrossAttention (silu-scored, masked) sharded over
8 NeuronCores.

Problem (full shapes):
    query/key/value: [2, 2048, 1024] f32, mask: [2, 1, 2048, 2048] int32
    Wq/Wk/Wv/Wo: [1024, 1024] f32, bq/bk/bv: [1024] f32
    out = silu(mask((q @ k.T) * scale)) @ v heads-merged @ Wo.T

Sharding: core c handles batch b = c // 4 and heads 4*(c%4) .. 4*(c%4)+3
(data parallel on B, tensor parallel on heads).  Each core computes a
row-parallel partial of the O-projection; the host sums the 4 partials per
batch.  No cross-device communication.

v2 design (vs the padded baseline):
  * score matmuls run the two heads of a pair CONCURRENTLY on the PE via
    row tiling (K=64 tiles at array rows 0-63 / 64-127, auto-derived from
    the operands' base partitions) -- no zero-padding of k, 2x PE win.
  * ctx matmuls pack the pair via col tiling (M=64 -> PSUM partitions
    0-63 / 64-127 of one shared accumulator) -- no M padding, and the
    accumulator IS the O-proj lhsT layout, so no scatter copies.
  * v is projected directly in [sk, dh] orientation (lhsT = x tile,
    rhs = Wv) -- kills the PE transposes + scatter of the baseline.
    v bias is folded in with a K=1 ones-matmul into the PSUM accumulator.
  * mask is resident in SBUF (loaded once, 8 MB) instead of streamed
    4x (32 MB); silu(masked) == mask-mul after silu since silu(0)=0 and
    the reference's -1e9 fill makes silu ~0 too.
  * mask applied AFTER silu, as bf16 x bf16 tensor_tensor (2x DVE mode).
  * SCALE folded into Wq/bq on the host.
  * attention walks sq in halves of 1024 so PSUM fits:
    sT pair 2x[128,1024] (4 banks) + acc [128,1024] (2) + pot 2x[128,512]
    (2) = 8 banks; O-projection of each half overlaps the next half's
    attention.
"""

import numpy as np
import ml_dtypes

B = 2
S = 2048
HID = 1024
HEADS = 16
DH = 64
N_CORES = 8
GROUPS = 4          # head-groups (cores per batch)
NH_LOC = HEADS // GROUPS  # 4 heads per core
DLOC = NH_LOC * DH        # 256 local features
SCALE = DH ** -0.5

F32 = np.float32
BF16 = ml_dtypes.bfloat16

_COMPILED = {}


def build_program():
    import concourse.bass as bass
    import concourse.tile as tile
    from concourse import bacc, mybir

    f32 = mybir.dt.float32
    bf16 = mybir.dt.bfloat16

    nc = bacc.Bacc("TRN2", target_bir_lowering=False, debug=False,
                   enable_asserts=False, num_devices=N_CORES)

    xq = nc.dram_tensor("xq", [HID, S], bf16, kind="ExternalInput").ap()
    xk = nc.dram_tensor("xk", [HID, S], bf16, kind="ExternalInput").ap()
    xv = nc.dram_tensor("xv", [HID, S], bf16, kind="ExternalInput").ap()
    mk = nc.dram_tensor("mk", [S, S], bf16, kind="ExternalInput").ap()
    wq = nc.dram_tensor("wq", [HID, DLOC], bf16, kind="ExternalInput").ap()
    wk = nc.dram_tensor("wk", [HID, DLOC], bf16, kind="ExternalInput").ap()
    wv = nc.dram_tensor("wv", [HID, DLOC], bf16, kind="ExternalInput").ap()
    wo = nc.dram_tensor("wo", [DLOC, HID], bf16, kind="ExternalInput").ap()
    bq = nc.dram_tensor("bq", [DLOC, 1], f32, kind="ExternalInput").ap()
    bk = nc.dram_tensor("bk", [DLOC, 1], f32, kind="ExternalInput").ap()
    bv = nc.dram_tensor("bv", [1, DLOC], bf16, kind="ExternalInput").ap()
    out1 = nc.dram_tensor("out1", [S, HID], bf16, kind="ExternalOutput").ap()

    SILU = mybir.ActivationFunctionType.Silu
    ADD = mybir.AluOpType.add

    with tile.TileContext(nc) as tc:
        with (
            tc.tile_pool(name="res", bufs=1) as res,
            tc.tile_pool(name="io", bufs=16) as io,
            tc.tile_pool(name="wp", bufs=3) as wp,
            # PSUM: sT 2x[128,1024] (4 banks) + acc 1x[128,1024] (2) +
            # pot 2x[128,512] (2) = 8 banks.  Phase A paccs ride the pot
            # slots; phase C pots do too (disjoint in time).
            tc.tile_pool(name="ps", bufs=2, space="PSUM") as ps,
            tc.tile_pool(name="attp", bufs=4) as attp,
            tc.tile_pool(name="oev", bufs=4) as oev,
        ):
            # ---- resident SBUF tensors ----
            qt = [res.tile([128, S], bf16, tag=f"qt{m}", name=f"qt{m}") for m in range(2)]
            kt2 = [res.tile([128, S], bf16, tag=f"kt{m}", name=f"kt{m}") for m in range(2)]
            ctxt = [res.tile([128, S], bf16, tag=f"ctxt{m}", name=f"ctxt{m}") for m in range(2)]
            v_sb = res.tile([128, 16 * DLOC], bf16, tag="vsb", name="v_sb")  # [sk%128, j*256+dh4]
            mk_sb = res.tile([128, 16 * S], bf16, tag="mk", name="mk_sb")    # [sk%128, j*2048+sq]
            wo_sb = [res.tile([128, HID], bf16, tag=f"wo{k}", name=f"wo_sb{k}") for k in range(2)]
            b_sb = {}
            for nm, srcb in (("bq", bq), ("bk", bk)):
                b_sb[nm] = [res.tile([128, 1], f32, tag=f"{nm}{m}", name=f"{nm}_sb{m}") for m in range(2)]
                for m in range(2):
                    nc.scalar.dma_start(out=b_sb[nm][m][:, :], in_=srcb[m * 128:(m + 1) * 128, :])
            bv_sb = res.tile([1, DLOC], bf16, tag="bv", name="bv_sb")
            nc.scalar.dma_start(out=bv_sb[:, :], in_=bv[:, :])
            ones = res.tile([1, 128], bf16, tag="ones", name="ones")
            nc.gpsimd.memset(ones[:, :], 1.0)
            scr = res.tile([1, 1], bf16, tag="scr", name="scr")
            # dummy silu: hoists the ACT table load out of the hot loop
            nc.scalar.activation(scr[:, :], ones[0:1, 0:1], SILU)
            for k in range(2):
                nc.scalar.dma_start(out=wo_sb[k][:, :], in_=wo[k * 128:(k + 1) * 128, :])

            # ---- input DMAs (two HWDGE queues: sync for k/q, scalar for rest) ----
            w_sb = {}
            for nm, w_ap in (("k", wk), ("q", wq), ("v", wv)):
                w_sb[nm] = wp.tile([128, 8 * DLOC], bf16, tag="w", name=f"w_{nm}")
                dmae = nc.sync if nm in ("k", "q") else nc.scalar
                dmae.dma_start(
                    out=w_sb[nm][:, :].rearrange("p (k m) -> p k m", k=8),
                    in_=w_ap.rearrange("(k p) m -> p k m", p=128),
                )
            xt = {}
            for nm, x_ap in (("k", xk), ("q", xq)):
                xt[nm] = []
                for k in range(8):
                    t = io.tile([128, S], bf16, tag="x", name=f"x_{nm}{k}")
                    nc.sync.dma_start(out=t[:, :], in_=x_ap[k * 128:(k + 1) * 128, :])
                    xt[nm].append(t)
            for j in range(4):
                nc.scalar.dma_start(out=mk_sb[:, j * S:(j + 1) * S], in_=mk[j * 128:(j + 1) * 128, :])
            xt["v"] = []
            for k in range(8):
                t = io.tile([128, S], bf16, tag="x", name=f"x_v{k}")
                nc.scalar.dma_start(out=t[:, :], in_=xv[k * 128:(k + 1) * 128, :])
                xt["v"].append(t)
            for j in range(4, 16):
                nc.scalar.dma_start(out=mk_sb[:, j * S:(j + 1) * S], in_=mk[j * 128:(j + 1) * 128, :])

            # ---- Phase A: projections ----
            def qk_proj(nm, dst, bias, nblocks):
                # both m per n-block so xk/xq tiles free up after one pass
                for n in nblocks:
                    pacc = {m: ps.tile([128, 512], f32, tag="pot", name=f"p{nm}{m}", bufs=2) for m in range(2)}
                    for k in range(8):
                        for m in range(2):
                            nc.tensor.matmul(
                                pacc[m][:, :],
                                lhsT=w_sb[nm][:, k * DLOC + m * 128: k * DLOC + (m + 1) * 128],
                                rhs=xt[nm][k][:, n * 512:(n + 1) * 512],
                                start=(k == 0), stop=(k == 7),
                            )
                    for m in range(2):
                        nc.vector.tensor_scalar(
                            out=dst[m][:, n * 512:(n + 1) * 512],
                            in0=pacc[m][:, :],
                            scalar1=bias[m][:, 0:1],
                            scalar2=None,
                            op0=ADD,
                        )

            def v_proj(jblocks):
                for j in jblocks:
                    pv = ps.tile([128, 512], f32, tag="pot", name=f"pv{j}", bufs=2)
                    nc.tensor.matmul(
                        pv[:, 0:DLOC], lhsT=ones[:, :], rhs=bv_sb[:, :],
                        start=True, stop=False,
                    )
                    for k in range(8):
                        nc.tensor.matmul(
                            pv[:, 0:DLOC],
                            lhsT=xt["v"][k][:, j * 128:(j + 1) * 128],
                            rhs=w_sb["v"][:, k * DLOC:(k + 1) * DLOC],
                            start=False, stop=(k == 7),
                        )
                    nc.vector.tensor_copy(
                        out=v_sb[:, j * DLOC:(j + 1) * DLOC], in_=pv[:, 0:DLOC])

            qk_proj("k", kt2, b_sb["bk"], range(4))
            qk_proj("q", qt, b_sb["bq"], range(2))
            v_proj(range(8))
            qk_proj("q", qt, b_sb["bq"], range(2, 4))
            v_proj(range(8, 16))

            # ---- Phase B+C: attention (sq halves), O-proj per half ----
            for half in range(2):
                sqo = half * 1024
                for t in range(2):
                    acc = ps.tile([128, 1024], f32, tag="acc", name=f"acc{half}{t}", bufs=1)
                    hsl = [slice(0, 64), slice(64, 128)]

                    def ctx_mms(j, attn):
                        for s2 in range(2):
                            for h2 in range(2):
                                h = 2 * t + h2
                                nc.tensor.matmul(
                                    acc[hsl[h2], s2 * 512:(s2 + 1) * 512],
                                    lhsT=v_sb[:, j * DLOC + h * 64: j * DLOC + (h + 1) * 64],
                                    rhs=attn[h2][:, s2 * 512:(s2 + 1) * 512],
                                    start=(j == 0), stop=(j == 15),
                                )

                    # software-pipelined: scores(j) are emitted BEFORE
                    # ctx(j-1) so the PE FIFO never blocks behind the
                    # silu->mask chain of the previous iteration.
                    attn_prev = None
                    for j in range(16):
                        # head pair: adjacent PE instructions on disjoint
                        # 64-row (scores) / 64-col (ctx) array tiles so the
                        # two heads' matmuls run CONCURRENTLY and the array
                        # reads as fully active (keeps HAM at 2.4 GHz).
                        sT = [ps.tile([128, 1024], f32, tag="sT", name="sT", bufs=2)
                              for _ in range(2)]
                        for s2 in range(2):
                            for h2 in range(2):
                                nc.tensor.matmul(
                                    sT[h2][:, s2 * 512:(s2 + 1) * 512],
                                    lhsT=kt2[t][hsl[h2], j * 128:(j + 1) * 128],
                                    rhs=qt[t][hsl[h2], sqo + s2 * 512: sqo + (s2 + 1) * 512],
                                    start=True, stop=True,
                                )
                        att = [attp.tile([128, 1024], bf16, tag="att", name="att")
                               for _ in range(2)]
                        attn = [attp.tile([128, 1024], bf16, tag="attn", name="attn")
                                for _ in range(2)]
                        for h2 in range(2):
                            nc.scalar.activation(att[h2][:, :], sT[h2][:, :], SILU)
                        for h2 in range(2):
                            nc.vector.tensor_mul(
                                out=attn[h2][:, :], in0=att[h2][:, :],
                                in1=mk_sb[:, j * S + sqo: j * S + sqo + 1024],
                            )
                        if attn_prev is not None:
                            ctx_mms(j - 1, attn_prev)
                        attn_prev = attn
                    ctx_mms(15, attn_prev)
                    nc.vector.tensor_copy(
                        out=ctxt[t][:, sqo:sqo + 1024], in_=acc[:, :])
                # O-projection for this half (overlaps next half's attention)
                for blk in range(half * 8, half * 8 + 8):
                    for n2 in range(2):
                        pot = ps.tile([128, 512], f32, tag="pot", name="pot", bufs=2)
                        for ki in range(2):
                            nc.tensor.matmul(
                                pot[:, :],
                                lhsT=ctxt[ki][:, blk * 128:(blk + 1) * 128],
                                rhs=wo_sb[ki][:, n2 * 512:(n2 + 1) * 512],
                                start=(ki == 0), stop=(ki == 1),
                            )
                        ev = oev.tile([128, 512], bf16, tag="oev", name="ev")
                        if half == 0 or n2 == 0:
                            nc.vector.tensor_copy(out=ev[:, :], in_=pot[:, :])
                        else:
                            nc.scalar.copy(out=ev[:, :], in_=pot[:, :])
                        nc.sync.dma_start(
                            out=out1[blk * 128:(blk + 1) * 128, n2 * 512:(n2 + 1) * 512],
                            in_=ev[:, :],
                        )

    nc.compile()
    return nc


def get_program():
    if "nc" not in _COMPILED:
        _COMPILED["nc"] = build_program()
    return _COMPILED["nc"]


def make_in_maps(query, key, value, mask, Wq, bq, Wk, bk, Wv, bv, Wo):
    """Host-side sharding/layout prep: one input map per core."""
    query = np.asarray(query, dtype=F32)
    key = np.asarray(key, dtype=F32)
    value = np.asarray(value, dtype=F32)
    mask = np.asarray(mask)
    in_maps = []
    maskT = [np.ascontiguousarray(mask[b, 0].T).astype(BF16) for b in range(B)]
    xqT = [np.ascontiguousarray(query[b].T).astype(BF16) for b in range(B)]
    xkT = [np.ascontiguousarray(key[b].T).astype(BF16) for b in range(B)]
    xvT = [np.ascontiguousarray(value[b].T).astype(BF16) for b in range(B)]
    for c in range(N_CORES):
        b = c // GROUPS
        g = c % GROUPS
        rs = slice(g * DLOC, (g + 1) * DLOC)
        in_maps.append({
            "xq": xqT[b],
            "xk": xkT[b],
            "xv": xvT[b],
            "mk": maskT[b],
            "wq": np.ascontiguousarray(SCALE * np.asarray(Wq, F32)[rs, :].T).astype(BF16),
            "wk": np.ascontiguousarray(np.asarray(Wk, F32)[rs, :].T).astype(BF16),
            "wv": np.ascontiguousarray(np.asarray(Wv, F32)[rs, :].T).astype(BF16),
            "wo": np.ascontiguousarray(np.asarray(Wo, F32)[:, rs].T).astype(BF16),
            "bq": (SCALE * np.asarray(bq, F32)[rs]).reshape(DLOC, 1),
            "bk": np.asarray(bk, F32)[rs].reshape(DLOC, 1),
            "bv": np.asarray(bv, F32)[rs].reshape(1, DLOC).astype(BF16),
        })
    return in_maps


def run_on_device(in_maps, trace=False, tmpdir=None):
    from concourse.bass_utils import run_bass_kernel_spmd
    nc = get_program()
    kwargs = {}
    if trace:
        kwargs.update(trace=True, tmpdir=tmpdir)
    return run_bass_kernel_spmd(nc, in_maps, list(range(N_CORES)), **kwargs)


def assemble_output(results):
    out = np.zeros((B, S, HID), dtype=F32)
    for c in range(N_CORES):
        out[c // GROUPS] += results[c]["out1"].astype(F32)
    return out


def kernel(query, key, value, mask, Wq, bq, Wk, bk, Wv, bv, Wo):
    in_maps = make_in_maps(query, key, value, mask, Wq, bq, Wk, bk, Wv, bv, Wo)
    res = run_on_device(in_maps)
    return assemble_output(res.results)


# revision 9
# speedup vs baseline: 1.6232x; 1.2777x over previous
"""Trainium2 Bass kernel for CrossAttention (silu-scored, masked) sharded over
8 NeuronCores.

Problem (full shapes):
    query/key/value: [2, 2048, 1024] f32, mask: [2, 1, 2048, 2048] int32
    Wq/Wk/Wv/Wo: [1024, 1024] f32, bq/bk/bv: [1024] f32
    out = silu(mask((q @ k.T) * scale)) @ v heads-merged @ Wo.T

Sharding: core c handles batch b = c // 4 and heads 4*(c%4) .. 4*(c%4)+3
(data parallel on B, tensor parallel on heads).  Each core computes a
row-parallel partial of the O-projection; the host sums the 4 partials per
batch.  No cross-device communication.

v2 design (vs the padded baseline):
  * score matmuls run the two heads of a pair CONCURRENTLY on the PE via
    row tiling (K=64 tiles at array rows 0-63 / 64-127, auto-derived from
    the operands' base partitions) -- no zero-padding of k, 2x PE win.
  * ctx matmuls pack the pair via col tiling (M=64 -> PSUM partitions
    0-63 / 64-127 of one shared accumulator) -- no M padding, and the
    accumulator IS the O-proj lhsT layout, so no scatter copies.
  * v is projected directly in [sk, dh] orientation (lhsT = x tile,
    rhs = Wv) -- kills the PE transposes + scatter of the baseline.
    v bias is folded in with a K=1 ones-matmul into the PSUM accumulator.
  * mask is resident in SBUF (loaded once, 8 MB) instead of streamed
    4x (32 MB); silu(masked) == mask-mul after silu since silu(0)=0 and
    the reference's -1e9 fill makes silu ~0 too.
  * mask applied AFTER silu, as bf16 x bf16 tensor_tensor (2x DVE mode).
  * SCALE folded into Wq/bq on the host.
  * attention walks sq in halves of 1024 so PSUM fits:
    sT pair 2x[128,1024] (4 banks) + acc [128,1024] (2) + pot 2x[128,512]
    (2) = 8 banks; O-projection of each half overlaps the next half's
    attention.
"""

import numpy as np
import ml_dtypes

B = 2
S = 2048
HID = 1024
HEADS = 16
DH = 64
N_CORES = 8
GROUPS = 4          # head-groups (cores per batch)
NH_LOC = HEADS // GROUPS  # 4 heads per core
DLOC = NH_LOC * DH        # 256 local features
SCALE = DH ** -0.5

F32 = np.float32
BF16 = ml_dtypes.bfloat16

_COMPILED = {}


def build_program():
    import concourse.bass as bass
    import concourse.tile as tile
    from concourse import bacc, mybir

    f32 = mybir.dt.float32
    bf16 = mybir.dt.bfloat16

    nc = bacc.Bacc("TRN2", target_bir_lowering=False, debug=False,
                   enable_asserts=False, num_devices=N_CORES)

    xq = nc.dram_tensor("xq", [HID, S], bf16, kind="ExternalInput").ap()
    xk = nc.dram_tensor("xk", [HID, S], bf16, kind="ExternalInput").ap()
    xv = nc.dram_tensor("xv", [HID, S], bf16, kind="ExternalInput").ap()
    mk = nc.dram_tensor("mk", [S, S], bf16, kind="ExternalInput").ap()
    wq = nc.dram_tensor("wq", [HID, DLOC], bf16, kind="ExternalInput").ap()
    wk = nc.dram_tensor("wk", [HID, DLOC], bf16, kind="ExternalInput").ap()
    wv = nc.dram_tensor("wv", [HID, DLOC], bf16, kind="ExternalInput").ap()
    wo = nc.dram_tensor("wo", [DLOC, HID], bf16, kind="ExternalInput").ap()
    bq = nc.dram_tensor("bq", [DLOC, 1], f32, kind="ExternalInput").ap()
    bk = nc.dram_tensor("bk", [DLOC, 1], f32, kind="ExternalInput").ap()
    bv = nc.dram_tensor("bv", [1, 2 * DLOC], bf16, kind="ExternalInput").ap()
    out1 = nc.dram_tensor("out1", [S, HID], bf16, kind="ExternalOutput").ap()

    SILU = mybir.ActivationFunctionType.Silu
    ADD = mybir.AluOpType.add

    with tile.TileContext(nc) as tc:
        with (
            tc.tile_pool(name="res", bufs=1) as res,
            tc.tile_pool(name="io", bufs=16) as io,
            tc.tile_pool(name="wp", bufs=3) as wp,
            # PSUM: sT 2x[128,1024] (4 banks) + acc 1x[128,1024] (2) +
            # pot 2x[128,512] (2) = 8 banks.  Phase A paccs ride the pot
            # slots; phase C pots do too (disjoint in time).
            tc.tile_pool(name="ps", bufs=2, space="PSUM") as ps,
            tc.tile_pool(name="attp", bufs=4) as attp,
            tc.tile_pool(name="oev", bufs=4) as oev,
        ):
            # ---- resident SBUF tensors ----
            qt = [res.tile([128, S], bf16, tag=f"qt{m}", name=f"qt{m}") for m in range(2)]
            kt2 = [res.tile([128, S], bf16, tag=f"kt{m}", name=f"kt{m}") for m in range(2)]
            ctxt = [res.tile([128, S], bf16, tag=f"ctxt{m}", name=f"ctxt{m}") for m in range(2)]
            v_sb = res.tile([128, 16 * DLOC], bf16, tag="vsb", name="v_sb")  # [sk%128, j*256+dh4]
            mk_sb = res.tile([128, 16 * S], bf16, tag="mk", name="mk_sb")    # [sk%128, j*2048+sq]
            wo_sb = [res.tile([128, HID], bf16, tag=f"wo{k}", name=f"wo_sb{k}") for k in range(2)]
            b_sb = {}
            for nm, srcb in (("bq", bq), ("bk", bk)):
                b_sb[nm] = [res.tile([128, 1], f32, tag=f"{nm}{m}", name=f"{nm}_sb{m}") for m in range(2)]
                for m in range(2):
                    nc.scalar.dma_start(out=b_sb[nm][m][:, :], in_=srcb[m * 128:(m + 1) * 128, :])
            bv_sb = res.tile([1, 2 * DLOC], bf16, tag="bv", name="bv_sb")
            nc.scalar.dma_start(out=bv_sb[:, :], in_=bv[:, :])
            ones = res.tile([1, 128], bf16, tag="ones", name="ones")
            nc.gpsimd.memset(ones[:, :], 1.0)
            scr = res.tile([1, 1], bf16, tag="scr", name="scr")
            # dummy silu: hoists the ACT table load out of the hot loop
            nc.scalar.activation(scr[:, :], ones[0:1, 0:1], SILU)
            for k in range(2):
                nc.scalar.dma_start(out=wo_sb[k][:, :], in_=wo[k * 128:(k + 1) * 128, :])

            # ---- input DMAs ----
            # All large loads go on ONE HWDGE queue (sync) in strict
            # priority order: a single InstDMACopy already fans out over
            # all 16 SDMA engines, and two queues with pending work
            # round-robin at packet granularity -- a second queue would
            # only steal bandwidth from the critical xk/xq stream.
            w_sb = {}

            def w_load(nm, w_ap):
                w_sb[nm] = wp.tile([128, 8 * DLOC], bf16, tag="w", name=f"w_{nm}")
                nc.sync.dma_start(
                    out=w_sb[nm][:, :].rearrange("p (k m) -> p k m", k=8),
                    in_=w_ap.rearrange("(k p) m -> p k m", p=128),
                )

            xt = {"k": [], "q": [], "v": []}

            def x_load(nm, x_ap):
                for k in range(8):
                    t = io.tile([128, S], bf16, tag="x", name=f"x_{nm}{k}", bufs=15)
                    nc.sync.dma_start(out=t[:, :], in_=x_ap[k * 128:(k + 1) * 128, :])
                    xt[nm].append(t)

            w_load("k", wk)
            x_load("k", xk)
            w_load("q", wq)
            x_load("q", xq)
            w_load("v", wv)
            x_load("v", xv)
            for j in range(16):
                nc.sync.dma_start(out=mk_sb[:, j * S:(j + 1) * S], in_=mk[j * 128:(j + 1) * 128, :])

            # ---- Phase A: projections, k-outer over 8 PSUM regions ----
            # The 8 accumulators cover all 8 PSUM banks; the k-loop fires 8
            # full-array matmuls the moment each x k-tile lands, so the PE
            # has continuous DMA-paced work from ~2us on (warms HAM early
            # and keeps it warm through phase A).
            def qk_proj(nm, dst, bias):
                pA = ps.tile([128, 512], f32, tag="pot", name=f"p{nm}a", bufs=2)
                pB = ps.tile([128, 512], f32, tag="pot", name=f"p{nm}b", bufs=2)
                sA = ps.tile([128, 1024], f32, tag="sT", name=f"s{nm}a", bufs=2)
                sB = ps.tile([128, 1024], f32, tag="sT", name=f"s{nm}b", bufs=2)
                aA = ps.tile([128, 1024], f32, tag="acc", name=f"a{nm}", bufs=1)
                regions = [
                    (0, 0, pA[:, :]), (0, 1, pB[:, :]),
                    (0, 2, sA[:, 0:512]), (0, 3, sA[:, 512:1024]),
                    (1, 0, sB[:, 0:512]), (1, 1, sB[:, 512:1024]),
                    (1, 2, aA[:, 0:512]), (1, 3, aA[:, 512:1024]),
                ]
                for k in range(8):
                    for m, n, reg in regions:
                        nc.tensor.matmul(
                            reg,
                            lhsT=w_sb[nm][:, k * DLOC + m * 128: k * DLOC + (m + 1) * 128],
                            rhs=xt[nm][k][:, n * 512:(n + 1) * 512],
                            start=(k == 0), stop=(k == 7),
                        )
                for m, n, reg in regions:
                    nc.vector.tensor_scalar(
                        out=dst[m][:, n * 512:(n + 1) * 512],
                        in0=reg, scalar1=bias[m][:, 0:1], scalar2=None, op0=ADD,
                    )

            def v_proj(jbase):
                sA = ps.tile([128, 1024], f32, tag="sT", name=f"vs{jbase}", bufs=2)
                aA = ps.tile([128, 1024], f32, tag="acc", name=f"va{jbase}", bufs=1)
                regs = ([sA[:, i * 256:(i + 1) * 256] for i in range(4)]
                        + [aA[:, i * 256:(i + 1) * 256] for i in range(4)])
                # v bias via K=1 ones-matmul (bias is per-COLUMN here).
                # ONE start=True write per PSUM BANK: start clears the
                # has_written bits bank-wide, so a second start=True into
                # the other half of a bank would wipe the first one's bias.
                for bank in (sA[:, 0:512], sA[:, 512:1024],
                             aA[:, 0:512], aA[:, 512:1024]):
                    nc.tensor.matmul(bank, lhsT=ones[:, :], rhs=bv_sb[:, :],
                                     start=True, stop=False,
                                     skip_group_check=True)
                for k in range(8):
                    for i, reg in enumerate(regs):
                        j = jbase + i
                        nc.tensor.matmul(
                            reg,
                            lhsT=xt["v"][k][:, j * 128:(j + 1) * 128],
                            rhs=w_sb["v"][:, k * DLOC:(k + 1) * DLOC],
                            start=False, stop=(k == 7),
                        )
                for i, reg in enumerate(regs):
                    nc.vector.tensor_copy(
                        out=v_sb[:, (jbase + i) * DLOC:(jbase + i + 1) * DLOC],
                        in_=reg)

            # ---- Phase B: one attention pass = (head pair t, sq half) ----
            hsl = [slice(0, 64), slice(64, 128)]

            def scores_mms(t, half, j):
                # two heads on disjoint 64-row PE tiles (auto tile_position
                # from base partitions) -- no K padding
                sqo = half * 1024
                sT = [ps.tile([128, 1024], f32, tag="sT", name="sT", bufs=2)
                      for _ in range(2)]
                for h2 in range(2):
                    for s2 in range(2):
                        nc.tensor.matmul(
                            sT[h2][:, s2 * 512:(s2 + 1) * 512],
                            lhsT=kt2[t][hsl[h2], j * 128:(j + 1) * 128],
                            rhs=qt[t][hsl[h2], sqo + s2 * 512: sqo + (s2 + 1) * 512],
                            start=True, stop=True,
                        )
                return sT

            def silu_mul(t, half, j, sT):
                sqo = half * 1024
                att = [attp.tile([128, 1024], bf16, tag="att", name="att")
                       for _ in range(2)]
                attn = [attp.tile([128, 1024], bf16, tag="attn", name="attn", bufs=6)
                        for _ in range(2)]
                for h2 in range(2):
                    nc.scalar.activation(att[h2][:, :], sT[h2][:, :], SILU)
                for h2 in range(2):
                    nc.vector.tensor_mul(
                        out=attn[h2][:, :], in0=att[h2][:, :],
                        in1=mk_sb[:, j * S + sqo: j * S + sqo + 1024],
                    )
                return attn

            def b_pass(t, half, hook=None):
                # depth-2 software pipeline: ctx(j-2) is emitted at iter j,
                # so the PE never waits on the silu->mask chain.
                sqo = half * 1024
                accbox = [None]
                pend = []

                def flush_one():
                    if accbox[0] is None:
                        accbox[0] = ps.tile([128, 1024], f32, tag="acc",
                                            name=f"acc{half}{t}", bufs=1)
                    jj, attn = pend.pop(0)
                    for s2 in range(2):
                        for h2 in range(2):
                            h = 2 * t + h2
                            nc.tensor.matmul(
                                accbox[0][hsl[h2], s2 * 512:(s2 + 1) * 512],
                                lhsT=v_sb[:, jj * DLOC + h * 64: jj * DLOC + (h + 1) * 64],
                                rhs=attn[h2][:, s2 * 512:(s2 + 1) * 512],
                                start=(jj == 0), stop=(jj == 15),
                            )

                for j in range(16):
                    sT = scores_mms(t, half, j)
                    pend.append((j, silu_mul(t, half, j, sT)))
                    if len(pend) > 2:
                        flush_one()
                    if hook is not None and j == 1:
                        hook()
                while pend:
                    flush_one()
                nc.vector.tensor_copy(
                    out=ctxt[t][:, sqo:sqo + 1024], in_=accbox[0][:, :])

            def o_proj(half):
                for blk in range(half * 8, half * 8 + 8):
                    for n2 in range(2):
                        pot = ps.tile([128, 512], f32, tag="pot", name="pot", bufs=2)
                        for ki in range(2):
                            nc.tensor.matmul(
                                pot[:, :],
                                lhsT=ctxt[ki][:, blk * 128:(blk + 1) * 128],
                                rhs=wo_sb[ki][:, n2 * 512:(n2 + 1) * 512],
                                start=(ki == 0), stop=(ki == 1),
                            )
                        ev = oev.tile([128, 512], bf16, tag="oev", name="ev")
                        if half == 0 or n2 == 0:
                            nc.vector.tensor_copy(out=ev[:, :], in_=pot[:, :])
                        else:
                            nc.scalar.copy(out=ev[:, :], in_=pot[:, :])
                        nc.sync.dma_start(
                            out=out1[blk * 128:(blk + 1) * 128, n2 * 512:(n2 + 1) * 512],
                            in_=ev[:, :],
                        )

            qk_proj("k", kt2, b_sb["bk"])
            qk_proj("q", qt, b_sb["bq"])
            # v-projection is emitted from inside the first attention pass
            # (after j=1) so B's first score/silu tiles get sT slots before
            # v claims them, without the ctx matmuls deadlocking on v_sb.
            b_pass(0, 0, hook=lambda: (v_proj(0), v_proj(8)))
            b_pass(1, 0)
            b_pass(0, 1)
            o_proj(0)   # overlaps B(h1); emitted after the next pass starts
            b_pass(1, 1)
            o_proj(1)

    nc.compile()
    return nc


def get_program():
    if "nc" not in _COMPILED:
        _COMPILED["nc"] = build_program()
    return _COMPILED["nc"]


def make_in_maps(query, key, value, mask, Wq, bq, Wk, bk, Wv, bv, Wo):
    """Host-side sharding/layout prep: one input map per core."""
    query = np.asarray(query, dtype=F32)
    key = np.asarray(key, dtype=F32)
    value = np.asarray(value, dtype=F32)
    mask = np.asarray(mask)
    in_maps = []
    maskT = [np.ascontiguousarray(mask[b, 0].T).astype(BF16) for b in range(B)]
    xqT = [np.ascontiguousarray(query[b].T).astype(BF16) for b in range(B)]
    xkT = [np.ascontiguousarray(key[b].T).astype(BF16) for b in range(B)]
    xvT = [np.ascontiguousarray(value[b].T).astype(BF16) for b in range(B)]
    for c in range(N_CORES):
        b = c // GROUPS
        g = c % GROUPS
        rs = slice(g * DLOC, (g + 1) * DLOC)
        in_maps.append({
            "xq": xqT[b],
            "xk": xkT[b],
            "xv": xvT[b],
            "mk": maskT[b],
            "wq": np.ascontiguousarray(SCALE * np.asarray(Wq, F32)[rs, :].T).astype(BF16),
            "wk": np.ascontiguousarray(np.asarray(Wk, F32)[rs, :].T).astype(BF16),
            "wv": np.ascontiguousarray(np.asarray(Wv, F32)[rs, :].T).astype(BF16),
            "wo": np.ascontiguousarray(np.asarray(Wo, F32)[:, rs].T).astype(BF16),
            "bq": (SCALE * np.asarray(bq, F32)[rs]).reshape(DLOC, 1),
            "bk": np.asarray(bk, F32)[rs].reshape(DLOC, 1),
            "bv": np.tile(np.asarray(bv, F32)[rs].reshape(1, DLOC), (1, 2)).astype(BF16),
        })
    return in_maps


def run_on_device(in_maps, trace=False, tmpdir=None):
    from concourse.bass_utils import run_bass_kernel_spmd
    nc = get_program()
    kwargs = {}
    if trace:
        kwargs.update(trace=True, tmpdir=tmpdir)
    return run_bass_kernel_spmd(nc, in_maps, list(range(N_CORES)), **kwargs)


def assemble_output(results):
    out = np.zeros((B, S, HID), dtype=F32)
    for c in range(N_CORES):
        out[c // GROUPS] += results[c]["out1"].astype(F32)
    return out


def kernel(query, key, value, mask, Wq, bq, Wk, bk, Wv, bv, Wo):
    in_maps = make_in_maps(query, key, value, mask, Wq, bq, Wk, bk, Wv, bv, Wo)
    res = run_on_device(in_maps)
    return assemble_output(res.results)


# revision 15
# speedup vs baseline: 1.6593x; 1.0223x over previous
"""Trainium2 Bass kernel for CrossAttention (silu-scored, masked) sharded over
8 NeuronCores.

Problem (full shapes):
    query/key/value: [2, 2048, 1024] f32, mask: [2, 1, 2048, 2048] int32
    Wq/Wk/Wv/Wo: [1024, 1024] f32, bq/bk/bv: [1024] f32
    out = silu(mask((q @ k.T) * scale)) @ v heads-merged @ Wo.T

Sharding: core c handles batch b = c // 4 and heads 4*(c%4) .. 4*(c%4)+3
(data parallel on B, tensor parallel on heads).  Each core computes a
row-parallel partial of the O-projection; the host sums the 4 partials per
batch.  No cross-device communication.

v2 design (vs the padded baseline):
  * score matmuls run the two heads of a pair CONCURRENTLY on the PE via
    row tiling (K=64 tiles at array rows 0-63 / 64-127, auto-derived from
    the operands' base partitions) -- no zero-padding of k, 2x PE win.
  * ctx matmuls pack the pair via col tiling (M=64 -> PSUM partitions
    0-63 / 64-127 of one shared accumulator) -- no M padding, and the
    accumulator IS the O-proj lhsT layout, so no scatter copies.
  * v is projected directly in [sk, dh] orientation (lhsT = x tile,
    rhs = Wv) -- kills the PE transposes + scatter of the baseline.
    v bias is folded in with a K=1 ones-matmul into the PSUM accumulator.
  * mask is resident in SBUF (loaded once, 8 MB) instead of streamed
    4x (32 MB); silu(masked) == mask-mul after silu since silu(0)=0 and
    the reference's -1e9 fill makes silu ~0 too.
  * mask applied AFTER silu, as bf16 x bf16 tensor_tensor (2x DVE mode).
  * SCALE folded into Wq/bq on the host.
  * attention walks sq in halves of 1024 so PSUM fits:
    sT pair 2x[128,1024] (4 banks) + acc [128,1024] (2) + pot 2x[128,512]
    (2) = 8 banks; O-projection of each half overlaps the next half's
    attention.
"""

import numpy as np
import ml_dtypes

B = 2
S = 2048
HID = 1024
HEADS = 16
DH = 64
N_CORES = 8
GROUPS = 4          # head-groups (cores per batch)
NH_LOC = HEADS // GROUPS  # 4 heads per core
DLOC = NH_LOC * DH        # 256 local features
SCALE = DH ** -0.5

F32 = np.float32
BF16 = ml_dtypes.bfloat16

_COMPILED = {}


def build_program():
    import concourse.bass as bass
    import concourse.tile as tile
    from concourse import bacc, mybir

    f32 = mybir.dt.float32
    bf16 = mybir.dt.bfloat16

    nc = bacc.Bacc("TRN2", target_bir_lowering=False, debug=False,
                   enable_asserts=False, num_devices=N_CORES)

    xq = nc.dram_tensor("xq", [HID, S], bf16, kind="ExternalInput").ap()
    xk = nc.dram_tensor("xk", [HID, S], bf16, kind="ExternalInput").ap()
    xv = nc.dram_tensor("xv", [HID, S], bf16, kind="ExternalInput").ap()
    mk = nc.dram_tensor("mk", [S, S], bf16, kind="ExternalInput").ap()
    wq = nc.dram_tensor("wq", [HID, DLOC], bf16, kind="ExternalInput").ap()
    wk = nc.dram_tensor("wk", [HID, DLOC], bf16, kind="ExternalInput").ap()
    wv = nc.dram_tensor("wv", [HID, DLOC], bf16, kind="ExternalInput").ap()
    wo = nc.dram_tensor("wo", [DLOC, HID], bf16, kind="ExternalInput").ap()
    bq = nc.dram_tensor("bq", [DLOC, 1], f32, kind="ExternalInput").ap()
    bk = nc.dram_tensor("bk", [DLOC, 1], f32, kind="ExternalInput").ap()
    bvb = nc.dram_tensor("bvb", [128, DLOC], f32, kind="ExternalInput").ap()
    out1 = nc.dram_tensor("out1", [S, HID], bf16, kind="ExternalOutput").ap()

    SILU = mybir.ActivationFunctionType.Silu
    ADD = mybir.AluOpType.add

    with tile.TileContext(nc) as tc:
        with (
            tc.tile_pool(name="res", bufs=1) as res,
            tc.tile_pool(name="io", bufs=16) as io,
            tc.tile_pool(name="wp", bufs=3) as wp,
            # PSUM: sT 2x[128,1024] (4 banks) + acc 1x[128,1024] (2) +
            # pot 2x[128,512] (2) = 8 banks.  Phase A paccs ride the pot
            # slots; phase C pots do too (disjoint in time).
            tc.tile_pool(name="ps", bufs=2, space="PSUM") as ps,
            tc.tile_pool(name="attp", bufs=4) as attp,
            tc.tile_pool(name="oev", bufs=4) as oev,
        ):
            # ---- resident SBUF tensors ----
            qt = [res.tile([128, S], bf16, tag=f"qt{m}", name=f"qt{m}") for m in range(2)]
            kt2 = [res.tile([128, S], bf16, tag=f"kt{m}", name=f"kt{m}") for m in range(2)]
            ctxt = [res.tile([128, S], bf16, tag=f"ctxt{m}", name=f"ctxt{m}") for m in range(2)]
            v_sb = res.tile([128, 16 * DLOC], bf16, tag="vsb", name="v_sb")  # [sk%128, j*256+dh4]
            mk_sb = res.tile([128, 16 * S], bf16, tag="mk", name="mk_sb")    # [sk%128, j*2048+sq]
            wo_sb = [res.tile([128, HID], bf16, tag=f"wo{k}", name=f"wo_sb{k}") for k in range(2)]
            b_sb = {}
            for nm, srcb in (("bq", bq), ("bk", bk)):
                b_sb[nm] = [res.tile([128, 1], f32, tag=f"{nm}{m}", name=f"{nm}_sb{m}") for m in range(2)]
                for m in range(2):
                    nc.scalar.dma_start(out=b_sb[nm][m][:, :], in_=srcb[m * 128:(m + 1) * 128, :])
            bvb_sb = res.tile([128, DLOC], f32, tag="bvb", name="bvb_sb")
            nc.scalar.dma_start(out=bvb_sb[:, :], in_=bvb[:, :])
            ones = res.tile([1, 128], bf16, tag="ones", name="ones")
            nc.gpsimd.memset(ones[:, :], 1.0)
            scr = res.tile([1, 1], bf16, tag="scr", name="scr")
            # dummy silu: hoists the ACT table load out of the hot loop
            nc.scalar.activation(scr[:, :], ones[0:1, 0:1], SILU)
            for k in range(2):
                nc.scalar.dma_start(out=wo_sb[k][:, :], in_=wo[k * 128:(k + 1) * 128, :])

            # ---- input DMAs ----
            # All large loads go on ONE HWDGE queue (sync) in strict
            # priority order: a single InstDMACopy already fans out over
            # all 16 SDMA engines, and two queues with pending work
            # round-robin at packet granularity -- a second queue would
            # only steal bandwidth from the critical xk/xq stream.
            w_sb = {}

            def w_load(nm, w_ap):
                w_sb[nm] = wp.tile([128, 8 * DLOC], bf16, tag="w", name=f"w_{nm}")
                nc.sync.dma_start(
                    out=w_sb[nm][:, :].rearrange("p (k m) -> p k m", k=8),
                    in_=w_ap.rearrange("(k p) m -> p k m", p=128),
                )

            xt = {"k": [], "q": [], "v": []}

            def x_load(nm, x_ap):
                for k in range(8):
                    t = io.tile([128, S], bf16, tag="x", name=f"x_{nm}{k}", bufs=15)
                    nc.sync.dma_start(out=t[:, :], in_=x_ap[k * 128:(k + 1) * 128, :])
                    xt[nm].append(t)

            w_load("k", wk)
            x_load("k", xk)
            w_load("q", wq)
            x_load("q", xq)
            w_load("v", wv)
            x_load("v", xv)
            for j in range(16):
                nc.sync.dma_start(out=mk_sb[:, j * S:(j + 1) * S], in_=mk[j * 128:(j + 1) * 128, :])

            # ---- Phase A: projections, k-outer over 8 PSUM regions ----
            # The 8 accumulators cover all 8 PSUM banks; the k-loop fires 8
            # full-array matmuls the moment each x k-tile lands, so the PE
            # has continuous DMA-paced work from ~2us on (warms HAM early
            # and keeps it warm through phase A).
            def qk_proj(nm, dst, bias):
                pA = ps.tile([128, 512], f32, tag="pot", name=f"p{nm}a", bufs=2)
                pB = ps.tile([128, 512], f32, tag="pot", name=f"p{nm}b", bufs=2)
                sA = ps.tile([128, 1024], f32, tag="sT", name=f"s{nm}a", bufs=2)
                sB = ps.tile([128, 1024], f32, tag="sT", name=f"s{nm}b", bufs=2)
                aA = ps.tile([128, 1024], f32, tag="acc", name=f"a{nm}", bufs=1)
                regions = [
                    (0, 0, pA[:, :]), (0, 1, pB[:, :]),
                    (0, 2, sA[:, 0:512]), (0, 3, sA[:, 512:1024]),
                    (1, 0, sB[:, 0:512]), (1, 1, sB[:, 512:1024]),
                    (1, 2, aA[:, 0:512]), (1, 3, aA[:, 512:1024]),
                ]
                for k in range(8):
                    for m, n, reg in regions:
                        nc.tensor.matmul(
                            reg,
                            lhsT=w_sb[nm][:, k * DLOC + m * 128: k * DLOC + (m + 1) * 128],
                            rhs=xt[nm][k][:, n * 512:(n + 1) * 512],
                            start=(k == 0), stop=(k == 7),
                        )
                for m, n, reg in regions:
                    nc.vector.tensor_scalar(
                        out=dst[m][:, n * 512:(n + 1) * 512],
                        in0=reg, scalar1=bias[m][:, 0:1], scalar2=None, op0=ADD,
                    )

            def v_proj(jbase):
                # 4 sk-tiles per pass, each accumulating in its OWN PSUM
                # bank (start=True clears has_written bank-wide, so two
                # accumulation groups must not share a bank).  Uses only
                # the pot/acc tags so phase-B score tiles never wait on v.
                # The per-column v bias is added by the DVE during evac.
                pA = ps.tile([128, 512], f32, tag="pot", name=f"vp{jbase}a", bufs=2)
                pB = ps.tile([128, 512], f32, tag="pot", name=f"vp{jbase}b", bufs=2)
                aA = ps.tile([128, 1024], f32, tag="acc", name=f"va{jbase}", bufs=1)
                regs = [pA[:, 0:DLOC], pB[:, 0:DLOC],
                        aA[:, 0:DLOC], aA[:, 512:512 + DLOC]]
                for k in range(8):
                    for i, reg in enumerate(regs):
                        j = jbase + i
                        nc.tensor.matmul(
                            reg,
                            lhsT=xt["v"][k][:, j * 128:(j + 1) * 128],
                            rhs=w_sb["v"][:, k * DLOC:(k + 1) * DLOC],
                            start=(k == 0), stop=(k == 7),
                        )
                for i, reg in enumerate(regs):
                    nc.vector.tensor_tensor(
                        out=v_sb[:, (jbase + i) * DLOC:(jbase + i + 1) * DLOC],
                        in0=reg, in1=bvb_sb[:, :], op=ADD)

            # ---- Phase B: one attention pass = (head pair t, sq half) ----
            hsl = [slice(0, 64), slice(64, 128)]

            def scores_mms(t, half, j):
                # two heads on disjoint 64-row PE tiles (auto tile_position
                # from base partitions) -- no K padding
                sqo = half * 1024
                sT = [ps.tile([128, 1024], f32, tag="sT", name="sT", bufs=2)
                      for _ in range(2)]
                for h2 in range(2):
                    for s2 in range(2):
                        nc.tensor.matmul(
                            sT[h2][:, s2 * 512:(s2 + 1) * 512],
                            lhsT=kt2[t][hsl[h2], j * 128:(j + 1) * 128],
                            rhs=qt[t][hsl[h2], sqo + s2 * 512: sqo + (s2 + 1) * 512],
                            start=True, stop=True,
                        )
                return sT

            def silu_mul(t, half, j, sT):
                sqo = half * 1024
                att = [attp.tile([128, 1024], bf16, tag="att", name="att")
                       for _ in range(2)]
                attn = [attp.tile([128, 1024], bf16, tag="attn", name="attn", bufs=6)
                        for _ in range(2)]
                for h2 in range(2):
                    nc.scalar.activation(att[h2][:, :], sT[h2][:, :], SILU)
                for h2 in range(2):
                    nc.vector.tensor_mul(
                        out=attn[h2][:, :], in0=att[h2][:, :],
                        in1=mk_sb[:, j * S + sqo: j * S + sqo + 1024],
                    )
                return attn

            def b_pass(t, half, hook=None):
                # depth-2 software pipeline: ctx(j-2) is emitted at iter j,
                # so the PE never waits on the silu->mask chain.
                sqo = half * 1024
                accbox = [None]
                pend = []

                def flush_one():
                    if accbox[0] is None:
                        accbox[0] = ps.tile([128, 1024], f32, tag="acc",
                                            name=f"acc{half}{t}", bufs=1)
                    jj, attn = pend.pop(0)
                    for s2 in range(2):
                        for h2 in range(2):
                            h = 2 * t + h2
                            nc.tensor.matmul(
                                accbox[0][hsl[h2], s2 * 512:(s2 + 1) * 512],
                                lhsT=v_sb[:, jj * DLOC + h * 64: jj * DLOC + (h + 1) * 64],
                                rhs=attn[h2][:, s2 * 512:(s2 + 1) * 512],
                                start=(jj == 0), stop=(jj == 15),
                            )

                for j in range(16):
                    sT = scores_mms(t, half, j)
                    pend.append((j, silu_mul(t, half, j, sT)))
                    if len(pend) > 2:
                        flush_one()
                    if hook is not None and j == 1:
                        hook()
                while pend:
                    flush_one()
                nc.vector.tensor_copy(
                    out=ctxt[t][:, sqo:sqo + 1024], in_=accbox[0][:, :])

            def o_proj(half):
                for blk in range(half * 8, half * 8 + 8):
                    for n2 in range(2):
                        # half 1 runs after the last silu, so the sT banks
                        # are free: alternate tags for a 4-deep rotation
                        tag = "pot" if (half == 0 or (blk + n2) % 2 == 0) else "sT"
                        pot = ps.tile([128, 512], f32, tag=tag, name="pot", bufs=2)
                        for ki in range(2):
                            nc.tensor.matmul(
                                pot[:, :],
                                lhsT=ctxt[ki][:, blk * 128:(blk + 1) * 128],
                                rhs=wo_sb[ki][:, n2 * 512:(n2 + 1) * 512],
                                start=(ki == 0), stop=(ki == 1),
                            )
                        ev = oev.tile([128, 512], bf16, tag="oev", name="ev")
                        if half == 0 or n2 == 0:
                            nc.vector.tensor_copy(out=ev[:, :], in_=pot[:, :])
                        else:
                            nc.scalar.copy(out=ev[:, :], in_=pot[:, :])
                        nc.sync.dma_start(
                            out=out1[blk * 128:(blk + 1) * 128, n2 * 512:(n2 + 1) * 512],
                            in_=ev[:, :],
                        )

            qk_proj("k", kt2, b_sb["bk"])
            qk_proj("q", qt, b_sb["bq"])
            # v-projection is emitted from inside the first attention pass
            # (after j=1) so B's first score/silu tiles get sT slots before
            # v claims them, without the ctx matmuls deadlocking on v_sb.
            b_pass(0, 0, hook=lambda: (v_proj(0), v_proj(4), v_proj(8), v_proj(12)))
            b_pass(1, 0)
            b_pass(0, 1)
            o_proj(0)   # overlaps B(h1); emitted after the next pass starts
            b_pass(1, 1)
            o_proj(1)

    nc.compile()
    return nc


def get_program():
    if "nc" not in _COMPILED:
        _COMPILED["nc"] = build_program()
    return _COMPILED["nc"]


def make_in_maps(query, key, value, mask, Wq, bq, Wk, bk, Wv, bv, Wo):
    """Host-side sharding/layout prep: one input map per core."""
    query = np.asarray(query, dtype=F32)
    key = np.asarray(key, dtype=F32)
    value = np.asarray(value, dtype=F32)
    mask = np.asarray(mask)
    in_maps = []
    maskT = [np.ascontiguousarray(mask[b, 0].T).astype(BF16) for b in range(B)]
    xqT = [np.ascontiguousarray(query[b].T).astype(BF16) for b in range(B)]
    xkT = [np.ascontiguousarray(key[b].T).astype(BF16) for b in range(B)]
    xvT = [np.ascontiguousarray(value[b].T).astype(BF16) for b in range(B)]
    for c in range(N_CORES):
        b = c // GROUPS
        g = c % GROUPS
        rs = slice(g * DLOC, (g + 1) * DLOC)
        in_maps.append({
            "xq": xqT[b],
            "xk": xkT[b],
            "xv": xvT[b],
            "mk": maskT[b],
            "wq": np.ascontiguousarray(SCALE * np.asarray(Wq, F32)[rs, :].T).astype(BF16),
            "wk": np.ascontiguousarray(np.asarray(Wk, F32)[rs, :].T).astype(BF16),
            "wv": np.ascontiguousarray(np.asarray(Wv, F32)[rs, :].T).astype(BF16),
            "wo": np.ascontiguousarray(np.asarray(Wo, F32)[:, rs].T).astype(BF16),
            "bq": (SCALE * np.asarray(bq, F32)[rs]).reshape(DLOC, 1),
            "bk": np.asarray(bk, F32)[rs].reshape(DLOC, 1),
            "bvb": np.broadcast_to(np.asarray(bv, F32)[rs].reshape(1, DLOC),
                                   (128, DLOC)).astype(F32).copy(),
        })
    return in_maps


def run_on_device(in_maps, trace=False, tmpdir=None):
    from concourse.bass_utils import run_bass_kernel_spmd
    nc = get_program()
    kwargs = {}
    if trace:
        kwargs.update(trace=True, tmpdir=tmpdir)
    return run_bass_kernel_spmd(nc, in_maps, list(range(N_CORES)), **kwargs)


def assemble_output(results):
    out = np.zeros((B, S, HID), dtype=F32)
    for c in range(N_CORES):
        out[c // GROUPS] += results[c]["out1"].astype(F32)
    return out


def kernel(query, key, value, mask, Wq, bq, Wk, bk, Wv, bv, Wo):
    in_maps = make_in_maps(query, key, value, mask, Wq, bq, Wk, bk, Wv, bv, Wo)
    res = run_on_device(in_maps)
    return assemble_output(res.results)
